# revision 15
# baseline (speedup 1.0000x reference)
"""ChemGCLayer Trainium2 kernel: 8-core SPMD, raw bass.

Strategy (dst-sharded GCN):
  - Nodes are sharded contiguously across 8 cores (12544 padded slots each).
  - Launch 1 (per core, its shard): nfeatsT = elu(W1^T@featsT+b1) (feature-major)
    and xw' rows = dinv[n] * (gc_in[n] @ Wg) (node-major, 1KB rows).
  - Host gathers xw' shards into one global table (plus a zeros pad row),
    replicates it to all cores.
  - Launch 2 (per core): for each 128-dst-node tile, gather the xw' rows of all
    incident edges (indirect DMA, int32 row ids), build a one-hot*dinv[dst]
    indicator [128 edges x 128 dst] on DVE, and matmul-scatter into PSUM
    (feature-major segment sum). Self-loops are extra edges. Then the final
    Wc matmul + elu produces outT; host transposes, concats feats, returns.
  - bg is folded into bc' = bc + bg@Wc[256:512] on host. norm = dinv_s*dinv_d
    is split: dinv_s folded into the xw' table, dinv_d into the indicator.
"""

import os
import sys

import numpy as np

N = 100000
E = 1600000
IN_F = 128
HID = 256
GCO = 256
FO = 256
NCORES = 8
P = 128
NPC = 12544            # nodes per core (98 tiles of 128)
NT = 98                # dst tiles per core
NTOT = NPC * NCORES    # 100352
ZROW = NTOT            # zeros pad row index in the xw table

_CACHE = {}


def _f32(x):
    return np.ascontiguousarray(x, dtype=np.float32)


def _host_prep(feats, edges, W1, b1, Wg, bg, Wc, bc):
    src = edges[0].astype(np.int64)
    dst = edges[1].astype(np.int64)

    deg = np.bincount(dst, minlength=N).astype(np.float32) + 1.0
    dinv = (1.0 / np.sqrt(deg)).astype(np.float32)
    dinv_pad = np.ones(NTOT, dtype=np.float32)
    dinv_pad[:N] = dinv

    # edge list incl. self loops, sharded by dst tile (contiguous shards)
    all_src = np.concatenate([src, np.arange(N, dtype=np.int64)])
    all_dst = np.concatenate([dst, np.arange(N, dtype=np.int64)])
    gtile = all_dst // P  # 0..781 ; cores own 98 consecutive tiles
    order = np.argsort(gtile, kind="stable")
    s_src = all_src[order]
    s_dst = all_dst[order]
    s_gt = gtile[order]
    NGT = NCORES * NT
    counts = np.bincount(s_gt, minlength=NGT)
    Q = int(np.ceil(counts.max() / P))
    W = Q * P
    starts = np.zeros(NGT, dtype=np.int64)
    starts[1:] = np.cumsum(counts)[:-1]
    pos = np.arange(len(s_src)) - starts[s_gt]

    offs = np.full((NGT, W), ZROW, dtype=np.int32)
    slot = np.full((NGT, W), -1000.0, dtype=np.float32)
    dind = np.zeros((NGT, W), dtype=np.float32)
    offs[s_gt, pos] = s_src.astype(np.int32)
    slot[s_gt, pos] = (s_dst % P).astype(np.float32)
    dind[s_gt, pos] = dinv[s_dst]

    # per-core [128, NT*Q] layouts: column = chunk (tile-major), row = edge slot
    def core_cols(a):
        # a: [NGT, W] -> per core [NT*Q, 128] -> T -> [128, NT*Q]
        out = []
        for c in range(NCORES):
            m = a[c * NT:(c + 1) * NT].reshape(NT * Q, P)
            out.append(np.ascontiguousarray(m.T))
        return out

    offs_c = core_cols(offs)
    slot_c = core_cols(slot)
    dind_c = core_cols(dind)

    # featsT / dinv cols per core
    featsT_c, dinvcol_c = [], []
    for c in range(NCORES):
        lo, hi = c * NPC, min((c + 1) * NPC, N)
        ft = np.zeros((P, NPC), dtype=np.float32)
        ft[:, :hi - lo] = feats[lo:hi].T
        featsT_c.append(ft)
        dv = dinv_pad[c * NPC:(c + 1) * NPC]
        dinvcol_c.append(np.ascontiguousarray(dv.reshape(NT, P).T.astype(np.float32)))

    w1 = _f32(W1)                                             # [128,256]
    wg = _f32(Wg.reshape(3, P, GCO).transpose(1, 0, 2).reshape(P, 3 * GCO))
    wc = _f32(Wc.reshape(4, P, FO).transpose(1, 0, 2).reshape(P, 4 * FO))
    b1c = _f32(b1.reshape(2, P).T)                            # [128,2]
    bcp = bc + bg @ Wc[HID:HID + GCO, :]
    bcc = _f32(bcp.reshape(2, P).T)                           # [128,2]
    return dict(Q=Q, offs=offs_c, slot=slot_c, dind=dind_c, featsT=featsT_c,
                dinvcol=dinvcol_c, w1=w1, wg=wg, wc=wc, b1c=b1c, bcc=bcc)


# --------------------------------------------------------------------------
# Launch 1: dense phase. Per core: nfeatsT [256,NPC], xw [NPC,256].
# --------------------------------------------------------------------------
def _build_launch1(bass, mybir):
    f32 = mybir.dt.float32
    A = mybir.AluOpType
    ACT = mybir.ActivationFunctionType
    nc = bass.Bass()
    d_ft = nc.declare_dram_parameter("featsT", [P, NPC], f32, isOutput=False)
    d_w1 = nc.declare_dram_parameter("w1", [P, 256], f32, isOutput=False)
    d_wg = nc.declare_dram_parameter("wg", [P, 768], f32, isOutput=False)
    d_b1 = nc.declare_dram_parameter("b1c", [P, 2], f32, isOutput=False)
    d_dv = nc.declare_dram_parameter("dinvc", [P, NT], f32, isOutput=False)
    d_nf = nc.declare_dram_parameter("nfT", [256, NPC], f32, isOutput=True)
    d_xw = nc.declare_dram_parameter("xw", [NPC, 256], f32, isOutput=True)

    sb = lambda n, s: nc.alloc_sbuf_tensor(n, s, f32)
    w1_sb = sb("w1s", [P, 256])
    wg_sb = sb("wgs", [P, 768])
    b1_sb = sb("b1s", [P, 2])
    dv_sb = sb("dvs", [P, NT])
    ft_sb = [sb(f"ft{i}", [P, 256]) for i in range(2)]
    nf_sb = [[sb(f"nf{i}{h}", [P, 256]) for h in range(2)] for i in range(2)]
    tmin = [[sb(f"tm{i}{h}", [P, 256]) for h in range(2)] for i in range(2)]
    texp = [[sb(f"te{i}{h}", [P, 256]) for h in range(2)] for i in range(2)]
    xw_sb = [sb(f"xw{i}", [P, 512]) for i in range(2)]

    ps = lambda n: nc.alloc_psum_tensor(n, [P, 512], f32)
    a1p = [[ps(f"a1{i}{h}") for h in range(2)] for i in range(2)]
    a2p = [ps(f"a2{i}") for i in range(2)]

    NA1 = 49  # tiles of 256 nodes

    with (nc.Block() as block,
          nc.semaphore("S_INIT") as S_INIT, nc.semaphore("S_LD") as S_LD,
          nc.semaphore("S_A1") as S_A1, nc.semaphore("S_M") as S_M,
          nc.semaphore("S_E") as S_E, nc.semaphore("S_NF") as S_NF,
          nc.semaphore("S_A2") as S_A2, nc.semaphore("S_XW") as S_XW,
          nc.semaphore("S_ST") as S_ST):

        @block.sync
        def _(sync):
            for pair in ((w1_sb, d_w1), (wg_sb, d_wg), (b1_sb, d_b1), (dv_sb, d_dv)):
                sync.dma_start(out=pair[0][:, :], in_=pair[1][:, :]).then_inc(S_INIT, 16)
            for i in range(2):
                sync.dma_start(out=ft_sb[i][:, :], in_=d_ft[:, i * 256:(i + 1) * 256]).then_inc(S_LD, 16)
            nst = 0
            for i in range(NA1):
                if i + 2 < NA1:
                    sync.wait_ge(S_A2, 2 * (i + 1))
                    sync.dma_start(out=ft_sb[i % 2][:, :],
                                   in_=d_ft[:, (i + 2) * 256:(i + 3) * 256]).then_inc(S_LD, 16)
                sync.wait_ge(S_NF, 2 * (i + 1))
                for h in range(2):
                    sync.dma_start(
                        out=bass.AP(d_nf, h * P * NPC + i * 256, [[NPC, P], [1, 256]]),
                        in_=nf_sb[i % 2][h][:, :]).then_inc(S_ST, 16)
                    nst += 1
                if i >= 1:
                    j = i - 1
                    sync.wait_ge(S_XW, 2 * (j + 1))
                    sync.dma_start(
                        out=bass.AP(d_xw, j * 256 * 256, [[256, P], [P * 256, 2], [1, 256]]),
                        in_=bass.AP(xw_sb[j % 2], 0, [[512, P], [256, 2], [1, 256]]),
                    ).then_inc(S_ST, 16)
                    nst += 1
            j = NA1 - 1
            sync.wait_ge(S_XW, 2 * (j + 1))
            sync.dma_start(
                out=bass.AP(d_xw, j * 256 * 256, [[256, P], [P * 256, 2], [1, 256]]),
                in_=bass.AP(xw_sb[j % 2], 0, [[512, P], [256, 2], [1, 256]]),
            ).then_inc(S_ST, 16)

        @block.tensor
        def _(tensor):
            tensor.wait_ge(S_INIT, 64)

            def a2(j):
                for u in range(2):
                    g = 2 * j + u
                    if g >= 2:
                        tensor.wait_ge(S_XW, g - 1)
                    for k in range(3):
                        lhsT = (nf_sb[j % 2][k][:, u * P:(u + 1) * P] if k < 2
                                else ft_sb[j % 2][:, u * P:(u + 1) * P])
                        mm = tensor.matmul(a2p[g % 2][:, :256], lhsT,
                                           wg_sb[:, k * 256:(k + 1) * 256],
                                           start=(k == 0), stop=(k == 2))
                    mm.then_inc(S_A2, 1)

            for i in range(NA1):
                tensor.wait_ge(S_LD, 16 * (i + 1))
                if i >= 2:
                    tensor.wait_ge(S_NF, 2 * (i - 1))
                for h in range(2):
                    tensor.matmul(a1p[i % 2][h][:, :256], w1_sb[:, h * P:(h + 1) * P],
                                  ft_sb[i % 2][:, :], start=True, stop=True).then_inc(S_A1, 1)
                if i >= 1:
                    tensor.wait_ge(S_NF, 2 * i)  # elu(i-1) fully done
                    a2(i - 1)
            tensor.wait_ge(S_NF, 2 * NA1)
            a2(NA1 - 1)

        @block.vector
        def _(vector):
            vector.wait_ge(S_INIT, 64)
            for i in range(NA1):
                for h in range(2):
                    vector.wait_ge(S_A1, 2 * i + h + 1)
                    if i >= 2:
                        vector.wait_ge(S_E, 2 * (i - 2) + h + 1)
                        vector.wait_ge(S_A2, 2 * (i - 1))
                        vector.wait_ge(S_ST, 16 * (3 * (i - 2) + 2))
                    pa = a1p[i % 2][h][:, :256]
                    vector.tensor_scalar(out=tmin[i % 2][h][:, :], in0=pa,
                                         scalar1=b1_sb[:, h:h + 1], scalar2=0.0,
                                         op0=A.add, op1=A.min).then_inc(S_M, 1)
                    vector.tensor_scalar(out=nf_sb[i % 2][h][:, :], in0=pa,
                                         scalar1=b1_sb[:, h:h + 1], scalar2=0.0,
                                         op0=A.add, op1=A.max)
                    vector.wait_ge(S_E, 2 * i + h + 1)
                    vector.tensor_tensor(out=nf_sb[i % 2][h][:, :], in0=nf_sb[i % 2][h][:, :],
                                         in1=texp[i % 2][h][:, :], op=A.add)
                    vector.tensor_scalar(out=nf_sb[i % 2][h][:, :], in0=nf_sb[i % 2][h][:, :],
                                         scalar1=-1.0, scalar2=None,
                                         op0=A.add).then_inc(S_NF, 1)
                if i >= 1:
                    j = i - 1
                    for u in range(2):
                        g = 2 * j + u
                        vector.wait_ge(S_A2, g + 1)
                        if j >= 2:
                            vector.wait_ge(S_ST, 16 * (3 * j - 1))
                        vector.tensor_scalar(out=xw_sb[j % 2][:, u * 256:(u + 1) * 256],
                                             in0=a2p[g % 2][:, :256],
                                             scalar1=dv_sb[:, g:g + 1], scalar2=None,
                                             op0=A.mult).then_inc(S_XW, 1)
            j = NA1 - 1
            for u in range(2):
                g = 2 * j + u
                vector.wait_ge(S_A2, g + 1)
                vector.tensor_scalar(out=xw_sb[j % 2][:, u * 256:(u + 1) * 256],
                                     in0=a2p[g % 2][:, :256],
                                     scalar1=dv_sb[:, g:g + 1], scalar2=None,
                                     op0=A.mult).then_inc(S_XW, 1)

        @block.scalar
        def _(scalar):
            for i in range(NA1):
                for h in range(2):
                    scalar.wait_ge(S_M, 2 * i + h + 1)
                    if i >= 2:
                        scalar.wait_ge(S_NF, 2 * (i - 2) + h + 1)
                    scalar.activation(out=texp[i % 2][h][:, :], in_=tmin[i % 2][h][:, :],
                                      func=ACT.Exp).then_inc(S_E, 1)

    return nc


# --------------------------------------------------------------------------
# Launch 2: gather + aggregate + final matmul. Per core out: outT [256, NPC].
# --------------------------------------------------------------------------
def _build_launch2(bass, mybir, Q):
    f32 = mybir.dt.float32
    i32 = mybir.dt.int32
    A = mybir.AluOpType
    ACTF = mybir.ActivationFunctionType
    nc = bass.Bass()
    NTQ = NT * Q
    d_tab = nc.declare_dram_parameter("xwtab", [NTOT + 1, GCO], f32, isOutput=False)
    d_nfT = nc.declare_dram_parameter("nfT", [256, NPC], f32, isOutput=False)
    d_off = nc.declare_dram_parameter("offs", [P, NTQ], i32, isOutput=False)
    d_slot = nc.declare_dram_parameter("dstslot", [P, NTQ], f32, isOutput=False)
    d_dind = nc.declare_dram_parameter("dinvd", [P, NTQ], f32, isOutput=False)
    d_wc = nc.declare_dram_parameter("wc", [P, 1024], f32, isOutput=False)
    d_bc = nc.declare_dram_parameter("bcc", [P, 2], f32, isOutput=False)
    d_iot = nc.declare_dram_parameter("iot", [P, Q * P], f32, isOutput=False)
    d_out = nc.declare_dram_parameter("outT", [256, NPC], f32, isOutput=True)

    sb = lambda n, s, dt=f32: nc.alloc_sbuf_tensor(n, s, dt)
    offs_sb = sb("offs_s", [P, NTQ], i32)
    slot_sb = sb("slot_s", [P, NTQ])
    dind_sb = sb("dind_s", [P, NTQ])
    wc_sb = sb("wc_s", [P, 1024])
    bc_sb = sb("bc_s", [P, 2])
    iotf = sb("iotf", [P, Q * P])
    gbuf = [sb(f"g{i}", [P, Q * GCO]) for i in range(2)]
    ind = [sb(f"ind{i}", [P, Q * P]) for i in range(2)]
    gcf = [[sb(f"gc{i}{h}", [P, P]) for h in range(2)] for i in range(2)]
    nfb = [sb(f"nfb{i}", [P, 2, 256]) for i in range(2)]
    outb = [[sb(f"ob{i}{h}", [P, 1024]) for h in range(2)] for i in range(2)]
    tmin = [[sb(f"tm{i}{h}", [P, P]) for h in range(2)] for i in range(2)]
    texp = [[sb(f"te{i}{h}", [P, P]) for h in range(2)] for i in range(2)]

    ps = lambda n: nc.alloc_psum_tensor(n, [P, 512], f32)
    psA = [[ps(f"pa{i}{h}") for h in range(2)] for i in range(2)]
    psC = [[ps(f"pc{i}{h}") for h in range(2)] for i in range(2)]

    NSUP = (NT + 7) // 8  # 13 supertiles for out stores

    with (nc.Block() as block,
          nc.semaphore("S_INIT") as S_INIT, nc.semaphore("S_G") as S_G,
          nc.semaphore("S_I") as S_I, nc.semaphore("S_A") as S_A,
          nc.semaphore("S_GC") as S_GC, nc.semaphore("S_C") as S_C,
          nc.semaphore("S_O") as S_O, nc.semaphore("S_NF") as S_NF,
          nc.semaphore("S_S") as S_S, nc.semaphore("S_M") as S_M,
          nc.semaphore("S_E") as S_E):

        @block.sync
        def _(sync):
            for dst_t, src_t in ((offs_sb, d_off), (slot_sb, d_slot),
                                 (dind_sb, d_dind), (wc_sb, d_wc), (bc_sb, d_bc),
                                 (iotf, d_iot)):
                sync.dma_start(out=dst_t[:, :], in_=src_t[:, :]).then_inc(S_INIT, 16)
            # nf loads (2 tiles per call) interleaved with out stores
            for j2 in range(49):
                if j2 >= 2:
                    sync.wait_ge(S_C, 2 * (j2 - 1))
                sync.dma_start(
                    out=bass.AP(nfb[j2 % 2], 0, [[512, P], [256, 2], [1, 256]]),
                    in_=bass.AP(d_nfT, j2 * 256, [[NPC, P], [P * NPC, 2], [1, 256]]),
                ).then_inc(S_NF, 16)
                if j2 >= 3 and (j2 - 3) % 4 == 0:
                    s = (j2 - 3) // 4
                    ntile = 8
                    sync.wait_ge(S_O, 2 * (s * 8 + ntile))
                    for h in range(2):
                        sync.dma_start(
                            out=bass.AP(d_out, h * P * NPC + s * 1024, [[NPC, P], [1, ntile * P]]),
                            in_=outb[s % 2][h][:, :ntile * P]).then_inc(S_S, 16)
            s = NSUP - 1
            ntile = NT - 8 * s
            sync.wait_ge(S_O, 2 * NT)
            for h in range(2):
                sync.dma_start(
                    out=bass.AP(d_out, h * P * NPC + s * 1024, [[NPC, P], [1, ntile * P]]),
                    in_=outb[s % 2][h][:, :ntile * P]).then_inc(S_S, 16)

        @block.gpsimd
        def _(gpsimd):
            gpsimd.wait_ge(S_INIT, 96)
            for t in range(NT):
                if t >= 2:
                    gpsimd.wait_ge(S_A, t - 1)
                for c in range(Q):
                    gpsimd.indirect_dma_start(
                        out=gbuf[t % 2][:, c * GCO:(c + 1) * GCO],
                        out_offset=None,
                        in_=d_tab[:, :],
                        in_offset=bass.IndirectOffsetOnAxis(
                            ap=offs_sb[:, t * Q + c:t * Q + c + 1], axis=0),
                    ).then_inc(S_G, 16)

        @block.tensor
        def _(tensor):
            tensor.wait_ge(S_INIT, 96)

            def phase_c(j):
                tensor.wait_ge(S_GC, j + 1)
                tensor.wait_ge(S_NF, 16 * (j // 2 + 1))
                if j >= 2:
                    tensor.wait_ge(S_O, 2 * (j - 2) + 2)
                for h in range(2):
                    for k in range(4):
                        rhs = (bass.AP(nfb[(j // 2) % 2], k * 256 + (j % 2) * P, [[512, P], [1, P]])
                               if k < 2 else gcf[j % 2][k - 2][:, :])
                        mm = tensor.matmul(psC[j % 2][h][:, :P],
                                           wc_sb[:, k * 256 + h * P:k * 256 + (h + 1) * P],
                                           rhs, start=(k == 0), stop=(k == 3))
                mm.then_inc(S_C, 1)

            for t in range(NT):
                tensor.wait_ge(S_I, t + 1)
                tensor.wait_ge(S_G, 16 * Q * (t + 1))
                if t >= 2:
                    tensor.wait_ge(S_GC, t - 1)
                for c in range(Q):
                    base = c * GCO
                    for h in range(2):
                        mm = tensor.matmul(psA[t % 2][h][:, :P],
                                           gbuf[t % 2][:, base + h * P:base + (h + 1) * P],
                                           ind[t % 2][:, c * P:(c + 1) * P],
                                           start=(c == 0), stop=(c == Q - 1))
                mm.then_inc(S_A, 1)
                if t >= 1:
                    phase_c(t - 1)
            phase_c(NT - 1)

        @block.vector
        def _(vector):
            vector.wait_ge(S_INIT, 96)

            def build_ind(t):
                b = t % 2
                i3 = bass.AP(ind[b], 0, [[Q * P, P], [P, Q], [1, P]])
                vector.tensor_tensor(out=i3, in0=bass.AP(iotf, 0, [[Q * P, P], [P, Q], [1, P]]),
                                     in1=slot_sb[:, t * Q:(t + 1) * Q].to_broadcast([P, Q, P]),
                                     op=A.is_equal)
                vector.tensor_tensor(out=i3, in0=i3,
                                     in1=dind_sb[:, t * Q:(t + 1) * Q].to_broadcast([P, Q, P]),
                                     op=A.mult).then_inc(S_I, 1)

            def elu(j):
                s8 = j // 8
                for h in range(2):
                    if j >= 2:
                        vector.wait_ge(S_E, 2 * (j - 2) + h + 1)
                    if s8 >= 2 and j % 8 == 0 and h == 0:
                        vector.wait_ge(S_S, 32 * (s8 - 1))
                    pc = psC[j % 2][h][:, :P]
                    ob = outb[s8 % 2][h][:, (j % 8) * P:(j % 8 + 1) * P]
                    vector.tensor_scalar(out=tmin[j % 2][h][:, :], in0=pc,
                                         scalar1=bc_sb[:, h:h + 1], scalar2=0.0,
                                         op0=A.add, op1=A.min).then_inc(S_M, 1)
                    vector.tensor_scalar(out=ob, in0=pc, scalar1=bc_sb[:, h:h + 1],
                                         scalar2=0.0, op0=A.add, op1=A.max)
                    vector.wait_ge(S_E, 2 * j + h + 1)
                    vector.tensor_tensor(out=ob, in0=ob, in1=texp[j % 2][h][:, :], op=A.add)
                    vector.tensor_scalar(out=ob, in0=ob, scalar1=-1.0, scalar2=None,
                                         op0=A.add, op1=A.bypass).then_inc(S_O, 1)

            build_ind(0)
            build_ind(1)
            for t in range(NT):
                vector.wait_ge(S_A, t + 1)
                if t >= 2:
                    vector.wait_ge(S_C, t - 1)
                vector.tensor_copy(out=gcf[t % 2][0][:, :], in_=psA[t % 2][0][:, :P])
                vector.tensor_copy(out=gcf[t % 2][1][:, :], in_=psA[t % 2][1][:, :P]).then_inc(S_GC, 1)
                if t + 2 < NT:
                    build_ind(t + 2)
                if t >= 1:
                    vector.wait_ge(S_C, t)
                    elu(t - 1)
            vector.wait_ge(S_C, NT)
            elu(NT - 1)

        @block.scalar
        def _(scalar):
            for j in range(NT):
                for h in range(2):
                    scalar.wait_ge(S_M, 2 * j + h + 1)
                    if j >= 2:
                        scalar.wait_ge(S_O, 2 * (j - 2) + h + 1)
                    scalar.activation(out=texp[j % 2][h][:, :], in_=tmin[j % 2][h][:, :],
                                      func=ACTF.Exp).then_inc(S_E, 1)

    return nc


def kernel(**inputs):
    sys.path.insert(0, "/opt/trn_rl_repo")
    from concourse import bass, mybir
    from concourse.bass_utils import run_bass_kernel_spmd

    feats = _f32(inputs["feats"])
    edges = inputs["edges"]
    batch = inputs["batch"]
    prep = _host_prep(feats, edges, inputs["W1"], inputs["b1"], inputs["Wg"],
                      inputs["bg"], inputs["Wc"], inputs["bc"])
    Q = prep["Q"]

    trace = bool(os.environ.get("KERNEL_TRACE"))
    if "L1" not in _CACHE:
        _CACHE["L1"] = _build_launch1(bass, mybir)
    nc1 = _CACHE["L1"]
    maps1 = [dict(featsT=prep["featsT"][c], w1=prep["w1"], wg=prep["wg"],
                  b1c=prep["b1c"], dinvc=prep["dinvcol"][c]) for c in range(NCORES)]
    import time as _time
    _t0 = _time.monotonic()
    res1 = run_bass_kernel_spmd(nc1, maps1, list(range(NCORES)), trace=trace)
    r1 = res1.results
    _CACHE["l1_wall_ns"] = int((_time.monotonic() - _t0) * 1e9)

    xwtab = np.zeros((NTOT + 1, GCO), dtype=np.float32)
    for c in range(NCORES):
        xwtab[c * NPC:(c + 1) * NPC] = r1[c]["xw"]

    if ("L2", Q) not in _CACHE:
        _CACHE[("L2", Q)] = _build_launch2(bass, mybir, Q)
    nc2 = _CACHE[("L2", Q)]
    iot = np.tile(np.arange(P, dtype=np.float32), Q)[None, :].repeat(P, 0)
    maps2 = [dict(xwtab=xwtab, nfT=r1[c]["nfT"], offs=prep["offs"][c],
                  dstslot=prep["slot"][c], dinvd=prep["dind"][c],
                  wc=prep["wc"], bcc=prep["bcc"], iot=iot) for c in range(NCORES)]
    _t0 = _time.monotonic()
    res2 = run_bass_kernel_spmd(nc2, maps2, list(range(NCORES)), trace=trace)
    r2 = res2.results
    _CACHE["l2_wall_ns"] = int((_time.monotonic() - _t0) * 1e9)
    if res1.exec_time_ns or res2.exec_time_ns:
        _CACHE["exec_time_ns"] = (res1.exec_time_ns or 0) + (res2.exec_time_ns or 0)
    else:
        _CACHE["exec_time_ns"] = _CACHE["l1_wall_ns"] + _CACHE["l2_wall_ns"]

    out_main = np.empty((N, FO), dtype=np.float32)
    for c in range(NCORES):
        lo, hi = c * NPC, min((c + 1) * NPC, N)
        out_main[lo:hi] = r2[c]["outT"].T[:hi - lo]
    out = np.concatenate([out_main, feats], axis=1)
    return (out, np.asarray(edges), np.asarray(batch))
